# revision 13
# baseline (speedup 1.0000x reference)
"""CrissCrossAttention3D Trainium2 kernel — fused single launch.

B=2, C=512, CQK=64, H=W=D=32, 8 NeuronCores. Core (b,g) owns the d-slab
dd in [8g, 8g+8) of batch b, voxels ordered (dd, h, w).

One SPMD launch does everything:
  A: q,k = Wqk @ x (f32, SBUF-resident), vT = x.T @ Wv.T (bf16 -> DRAM)
  AllGather (groups of 4 = one batch): k and vT along d -> d-major layouts
  Per axis (H local / W local / D gathered): per 4-line pack
     energy e[l,q] = K_line.T @ Q_line into diag psum blocks (tile_position)
     exp on ACT -> block-diagonal bf16 stationary (H: diagonal sign-flipped)
     agg U[q,c] = e.T @ V_pack, line sums via ones-column matmul
Host: decode line layouts, sig = sH+sW+sD, y = x + gamma*(UH+UW+UD)/sig.
The H diag sign flip makes unmasked 3-axis sums exactly equal the
reference's (-inf-masked H/D diag) softmax: -1+1+1 = +1 self term.
"""

import numpy as np
import ml_dtypes

import concourse.bass as bass
from concourse import bacc
import concourse.tile as tile
from concourse import mybir

BF16 = ml_dtypes.bfloat16
B, C, H, W, D = 2, 512, 32, 32, 32
CQK = 64
NCORES = 8
G = 4
DS = D // G     # 8
NV = 8192       # voxels per core
f32 = mybir.dt.float32
f32r = mybir.dt.float32r
bf16 = mybir.dt.bfloat16

_cache = {}


def build_fused(do_coll=True, do_h=True, do_w=True, do_d=True):
    nc = bacc.Bacc()
    x_in = nc.declare_dram_parameter("x", [4, 128, NV], f32r, isOutput=False)
    wqk_in = nc.declare_dram_parameter("wqk", [4, 128, 128], f32r, isOutput=False)
    wv_in = nc.declare_dram_parameter("wv", [4, 128, 512], f32r, isOutput=False)
    bqk_in = nc.declare_dram_parameter("bqk", [128, 1], f32, isOutput=False)
    bvb_in = nc.declare_dram_parameter("bvb", [128, 512], f32, isOutput=False)
    sgn_in = nc.declare_dram_parameter("sgn", [128, 32], bf16, isOutput=False)
    u_out = nc.declare_dram_parameter("u", [3, 64, 128, 512], bf16, isOutput=True)
    s_out = nc.declare_dram_parameter("s", [3, 8192], f32, isOutput=True)

    groups = [[0, 1, 2, 3], [4, 5, 6, 7]]

    with tile.TileContext(nc) as tc:
        with (
            tc.tile_pool(name="per", bufs=1) as per,
            tc.tile_pool(name="drp", bufs=1, space="DRAM") as drp,
        ):
            qk_sb = per.tile([128, NV], f32, tag="qk")
            k_loc = per.tile([64, NV], f32, tag="kloc")
            sums_h = per.tile([128, 64], f32, tag="sh")
            sums_w = per.tile([128, 64], f32, tag="sw")
            sums_d = per.tile([32, 256], f32, tag="sd")
            ones_sb = per.tile([128, 1], bf16, tag="ones")
            sgn_sb = per.tile([128, 32], bf16, tag="sgn")
            bqk_sb = per.tile([128, 1], f32, tag="bqk")
            bvb_sb = per.tile([128, 512], f32, tag="bvb")

            vt_dram = drp.tile([NV, 512], bf16, tag="vt")
            kb_dram = drp.tile([64, NV], f32, tag="kb")
            kg_dram = drp.tile([4, 64, NV], f32, tag="kg")
            vg_dram = drp.tile([4 * NV, 512], bf16, tag="vg")

            nc.vector.memset(ones_sb[:], 1.0)
            nc.sync.dma_start(sgn_sb[:], sgn_in[:])
            nc.sync.dma_start(bqk_sb[:], bqk_in[:])
            nc.sync.dma_start(bvb_sb[:], bvb_in[:])

            # ---------------- phase A: projections ----------------
            with (
                tc.tile_pool(name="aw", bufs=1) as aw,
                tc.tile_pool(name="ax", bufs=3) as ax,
                tc.tile_pool(name="aev", bufs=4) as aev,
                tc.tile_pool(name="aps", bufs=2, space="PSUM") as aps,
            ):
                wqk_sb = aw.tile([128, 512], f32r, tag="wqk")
                wv_sb = aw.tile([128, 2048], f32r, tag="wv")
                for ci in range(4):
                    nc.sync.dma_start(wqk_sb[:, ci * 128:(ci + 1) * 128],
                                      wqk_in[ci])
                    nc.sync.dma_start(wv_sb[:, ci * 512:(ci + 1) * 512],
                                      wv_in[ci])
                for nb in range(16):
                    xt = ax.tile([128, 2048], f32r, tag="x")
                    for ci in range(4):
                        nc.sync.dma_start(xt[:, ci * 512:(ci + 1) * 512],
                                          x_in[ci, :, nb * 512:(nb + 1) * 512])
                    ps_qk = aps.tile([128, 512], f32, tag="psqk")
                    for ci in range(4):
                        nc.tensor.matmul(ps_qk[:],
                                         wqk_sb[:, ci * 128:(ci + 1) * 128],
                                         xt[:, ci * 512:(ci + 1) * 512],
                                         start=(ci == 0), stop=(ci == 3))
                    nc.vector.tensor_scalar_add(
                        qk_sb[:, nb * 512:(nb + 1) * 512], ps_qk[:],
                        bqk_sb[:, 0:1])
                    for sub in range(4):
                        ps_v = aps.tile([128, 512], f32, tag="psv")
                        for ci in range(4):
                            nc.tensor.matmul(
                                ps_v[:],
                                xt[:, ci * 512 + sub * 128:
                                   ci * 512 + (sub + 1) * 128],
                                wv_sb[:, ci * 512:(ci + 1) * 512],
                                start=(ci == 0), stop=(ci == 3))
                        v_sb = aev.tile([128, 512], bf16, tag="v")
                        nc.vector.tensor_tensor(v_sb[:], ps_v[:], bvb_sb[:],
                                                op=mybir.AluOpType.add)
                        r0 = (nb * 4 + sub) * 128
                        nc.sync.dma_start(vt_dram[r0:r0 + 128, :], v_sb[:])

                nc.sync.dma_start(kb_dram[:], qk_sb[64:128, :])
                nc.sync.dma_start(k_loc[:], qk_sb[64:128, :])

            # ---------------- collectives ----------------
            if do_coll:
                nc.gpsimd.collective_compute(
                    "AllGather", mybir.AluOpType.bypass, replica_groups=groups,
                    ins=[kb_dram[:].opt()], outs=[kg_dram[:].opt()])
                nc.gpsimd.collective_compute(
                    "AllGather", mybir.AluOpType.bypass, replica_groups=groups,
                    ins=[vt_dram[:].opt()], outs=[vg_dram[:].opt()])

            # ---------------- phases B/C ----------------
            with (
                tc.tile_pool(name="kf", bufs=1) as kf,
                tc.tile_pool(name="bv", bufs=4) as bvp,
                tc.tile_pool(name="bev", bufs=4) as bev,
                tc.tile_pool(name="est", bufs=1) as estp,
                tc.tile_pool(name="eps", bufs=2, space="PSUM") as eps,
                tc.tile_pool(name="bps", bufs=2, space="PSUM") as bps,
                tc.tile_pool(name="sps", bufs=2, space="PSUM") as sps,
            ):
                k_full = kf.tile([64, 4 * NV], f32, tag="kfull")
                if do_coll:
                    for g in range(4):
                        nc.sync.dma_start(k_full[:, g * NV:(g + 1) * NV],
                                          kg_dram[g])
                NEB = 3
                eh = [estp.tile([128, 128], bf16, tag=f"eh{i}", name=f"eh{i}")
                      for i in range(NEB)]
                ew = [estp.tile([128, 128], bf16, tag=f"ew{i}", name=f"ew{i}")
                      for i in range(NEB)]
                ed = [estp.tile([128, 32], bf16, tag=f"ed{i}", name=f"ed{i}")
                      for i in range(NEB)]
                for t in eh + ew + ed:
                    nc.vector.memset(t[:], 0.0)

                q_ap = qk_sb[0:64, :]
                q_hw = q_ap.rearrange("c (dd h w) -> c dd h w",
                                      dd=8, h=32, w=32)
                k_hw = k_loc[:, :].rearrange("c (dd h w) -> c dd h w",
                                             dd=8, h=32, w=32)
                q_d = q_ap.rearrange("c (dd hw) -> c dd hw", dd=8)
                k_d = k_full[:, :].rearrange("c (d hw) -> c d hw", d=32)
                vt_h = vt_dram[:].rearrange("(dd h w) c -> dd w h c",
                                            dd=8, h=32, w=32)
                vt_w = vt_dram[:].rearrange("(dd h w) c -> dd h w c",
                                            dd=8, h=32, w=32)
                vg_d = vg_dram[:].rearrange("(d hw) c -> hw d c", d=32)

                # ---- H and W axes (local) ----
                axes = ([(0, "h")] if do_h else []) + ([(1, "w")] if do_w else [])
                for ax_i, axname in axes:
                    sums_sb = sums_h if ax_i == 0 else sums_w
                    ebufs = eh if ax_i == 0 else ew
                    for p in range(64):
                        dd, fp = p // 8, p % 8
                        pse = eps.tile([128, 128], f32, tag="pse",
                                       name=f"pse{axname}{p}")
                        for j in range(4):
                            f = 4 * fp + j
                            if ax_i == 0:
                                kst = k_hw[:, dd, :, f]
                                qmv = q_hw[:, dd, :, f]
                            else:
                                kst = k_hw[:, dd, f, :]
                                qmv = q_hw[:, dd, f, :]
                            nc.tensor.matmul(
                                pse[32 * j:32 * j + 32, 32 * j:32 * j + 32],
                                kst, qmv, start=True, stop=True,
                                tile_position=(0, 32 * j))
                        e = ebufs[p % NEB]
                        for j in range(4):
                            blk = (slice(32 * j, 32 * j + 32),
                                   slice(32 * j, 32 * j + 32))
                            nc.scalar.activation(
                                e[blk], pse[blk],
                                mybir.ActivationFunctionType.Exp)
                            if ax_i == 0:
                                nc.vector.tensor_tensor(
                                    e[blk], e[blk],
                                    sgn_sb[32 * j:32 * j + 32, :],
                                    op=mybir.AluOpType.mult)
                        v_t = bvp.tile([128, 512], bf16, tag="v",
                                       name=f"v{axname}{p}")
                        for j in range(4):
                            src = (vt_h[dd, 4 * fp + j] if ax_i == 0
                                   else vt_w[dd, 4 * fp + j])
                            nc.sync.dma_start(v_t[32 * j:32 * j + 32, :], src)
                        psu = bps.tile([128, 512], f32, tag="psu",
                                       name=f"psu{axname}{p}")
                        nc.tensor.matmul(psu[:], e[:], v_t[:],
                                         start=True, stop=True)
                        pss = sps.tile([128, 1], f32, tag="pss",
                                       name=f"pss{axname}{p}")
                        nc.tensor.matmul(pss[:], e[:], ones_sb[:],
                                         start=True, stop=True)
                        o_t = bev.tile([128, 512], bf16, tag="o",
                                       name=f"o{axname}{p}")
                        nc.scalar.activation(o_t[:], psu[:],
                                             mybir.ActivationFunctionType.Copy)
                        nc.sync.dma_start(u_out[ax_i, p], o_t[:])
                        nc.scalar.activation(sums_sb[:, p:p + 1], pss[:],
                                             mybir.ActivationFunctionType.Copy)
                    nc.sync.dma_start(
                        s_out[ax_i].rearrange("(p c) -> p c", p=128),
                        sums_sb[:])

                # ---- D axis (gathered) ----
                for p in range(256 if do_d else 0):
                    hq, wp = p // 8, p % 8
                    hw0 = hq * 32 + 4 * wp
                    pse = eps.tile([128, 128], f32, tag="pse",
                                   name=f"psed{p}")
                    for j in range(4):
                        hw = hw0 + j
                        nc.tensor.matmul(
                            pse[32 * j:32 * j + 32, 8 * j:8 * j + 8],
                            k_d[:, :, hw], q_d[:, :, hw],
                            start=True, stop=True,
                            tile_position=(0, 32 * j))
                    e = ed[p % NEB]
                    for j in range(4):
                        blk = (slice(32 * j, 32 * j + 32),
                               slice(8 * j, 8 * j + 8))
                        nc.scalar.activation(e[blk], pse[blk],
                                             mybir.ActivationFunctionType.Exp)
                    v_t = bvp.tile([128, 512], bf16, tag="v", name=f"vd{p}")
                    for j in range(4):
                        nc.sync.dma_start(v_t[32 * j:32 * j + 32, :],
                                          vg_d[hw0 + j])
                    psu = bps.tile([128, 512], f32, tag="psu", name=f"psud{p}")
                    nc.tensor.matmul(psu[0:32, :], e[:], v_t[:],
                                     start=True, stop=True)
                    pss = sps.tile([128, 1], f32, tag="pss", name=f"pssd{p}")
                    nc.tensor.matmul(pss[0:32, :], e[:], ones_sb[:],
                                     start=True, stop=True)
                    o_t = bev.tile([128, 512], bf16, tag="o", name=f"od{p}")
                    nc.scalar.activation(o_t[0:32, :], psu[0:32, :],
                                         mybir.ActivationFunctionType.Copy)
                    nc.sync.dma_start(
                        u_out[2, p // 4, (p % 4) * 32:(p % 4) * 32 + 32, :],
                        o_t[0:32, :])
                    nc.scalar.activation(sums_d[:, p:p + 1], pss[0:32, :],
                                         mybir.ActivationFunctionType.Copy)
                if do_d:
                    nc.sync.dma_start(
                        s_out[2].rearrange("(p c) -> p c", p=32), sums_d[:])
    return nc


def _get(name, builder):
    if name not in _cache:
        nc = builder()
        nc.finalize()
        _cache[name] = nc
    return _cache[name]


class _Runner:
    """jit-once PJRT runner for a prebuilt Bass module across 8 cores."""

    def __init__(self, nc):
        import jax
        from jax.experimental.shard_map import shard_map
        from jax.sharding import Mesh, PartitionSpec
        from concourse import bass2jax, mybir as _mb
        bass2jax.install_neuronx_cc_hook()
        self.nc = nc
        pname = nc.partition_id_tensor.name if nc.partition_id_tensor else None
        in_names, out_names, out_avals = [], [], []
        for alloc in nc.m.functions[0].allocations:
            if not isinstance(alloc, _mb.MemoryLocationSet):
                continue
            name = alloc.memorylocations[0].name
            if alloc.kind == "ExternalInput":
                if name != pname:
                    in_names.append(name)
            elif alloc.kind == "ExternalOutput":
                shape = tuple(alloc.tensor_shape)
                dt_np = _mb.dt.np(alloc.dtype)
                out_names.append(name)
                out_avals.append(jax.core.ShapedArray(shape, dt_np))
        self.in_names, self.out_names, self.out_avals = in_names, out_names, out_avals
        n_params = len(in_names)
        all_in = list(in_names) + list(out_names) + ([pname] if pname else [])

        def _body(*args):
            ops = list(args)
            if pname is not None:
                ops.append(bass2jax.partition_id_tensor())
            outs = bass2jax._bass_exec_p.bind(
                *ops, out_avals=tuple(out_avals), in_names=tuple(all_in),
                out_names=tuple(out_names), lowering_input_output_aliases=(),
                sim_require_finite=True, sim_require_nnan=True, nc=nc)
            return tuple(outs)

        devices = jax.devices()[:NCORES]
        mesh = Mesh(np.array(devices), ("core",))
        self.mesh = mesh
        n_io = n_params + len(out_names)
        self.donate = tuple(range(n_params, n_io))
        self.sharded = jax.jit(
            shard_map(_body, mesh=mesh,
                      in_specs=(PartitionSpec("core"),) * n_io,
                      out_specs=(PartitionSpec("core"),) * len(out_names),
                      check_rep=False),
            donate_argnums=self.donate, keep_unused=True)

    def _zeros(self):
        return [np.zeros((NCORES * a.shape[0], *a.shape[1:]), a.dtype)
                for a in self.out_avals]

    def __call__(self, in_maps):
        concat = [np.concatenate([np.asarray(m[n]) for m in in_maps], axis=0)
                  for n in self.in_names]
        arrs = self.sharded(*concat, *self._zeros())
        out = [{n: np.asarray(arrs[i]).reshape(NCORES, *self.out_avals[i].shape)[c]
                for i, n in enumerate(self.out_names)} for c in range(NCORES)]
        return out, (concat,)

    def bench(self, concat, n=3):
        import time, jax
        from jax.sharding import NamedSharding, PartitionSpec
        sh = NamedSharding(self.mesh, PartitionSpec("core"))
        dev_in = [jax.device_put(c, sh) for c in concat]
        jax.block_until_ready(dev_in)
        ts = []
        for _ in range(n):
            zs = [jax.device_put(z, sh) for z in self._zeros()]
            jax.block_until_ready(zs)
            t0 = time.perf_counter()
            arrs = self.sharded(*dev_in, *zs)
            jax.block_until_ready(arrs)
            ts.append(time.perf_counter() - t0)
        return min(ts)


class _RunRes:
    def __init__(self, results, exec_time_ns):
        self.results = results
        self.exec_time_ns = exec_time_ns


def _run(nc, in_maps, trace=False):
    import os
    key = id(nc)
    if key not in _cache:
        _cache[key] = _Runner(nc)
    runner = _cache[key]
    results, (concat,) = runner(in_maps)
    t = None
    if os.environ.get("BENCH"):
        t = int(runner.bench(concat, int(os.environ["BENCH"])) * 1e9)
    return _RunRes(results, t)


# --------------------------------------------------------------------------
# host orchestration
# --------------------------------------------------------------------------
def kernel(x, Wq, bq, Wk, bk, Wv, bv, gamma, _trace=False, _times=None):
    x = np.asarray(x, np.float32)
    Wq = np.asarray(Wq, np.float32); bq = np.asarray(bq, np.float32)
    Wk = np.asarray(Wk, np.float32); bk = np.asarray(bk, np.float32)
    Wv = np.asarray(Wv, np.float32); bv = np.asarray(bv, np.float32)
    gam = float(np.asarray(gamma))

    wqk = np.concatenate([Wq.T, Wk.T], axis=1).reshape(4, 128, 128)
    wv = np.ascontiguousarray(Wv.T).reshape(4, 128, 512)
    bqk = np.concatenate([bq, bk]).reshape(128, 1)
    bvb = np.broadcast_to(bv[None, :], (128, 512)).copy()
    sgn = np.ones((128, 32), np.float32)
    for j in range(4):
        np.fill_diagonal(sgn[32 * j:32 * j + 32], -1.0)
    sgn = sgn.astype(BF16)

    in_maps = []
    for core in range(NCORES):
        b, g = divmod(core, G)
        sl = slice(g * DS, (g + 1) * DS)
        xc = np.ascontiguousarray(
            x[b][:, :, :, sl].transpose(0, 3, 1, 2)).reshape(4, 128, NV)
        in_maps.append({"x": xc, "wqk": wqk, "wv": wv, "bqk": bqk,
                        "bvb": bvb, "sgn": sgn})

    r = _run(_get("fused", build_fused), in_maps, trace=_trace)
    if _times is not None:
        _times.append(r.exec_time_ns)

    y = np.empty_like(x)
    for b in range(B):
        acc = np.empty((D, H, W, C), np.float32)      # (d, h, w, c)
        sig_all = np.empty((D, H, W), np.float32)
        for g in range(G):
            core = b * G + g
            u = r.results[core]["u"].astype(np.float32)   # [3, 64, 128, 512]
            s = r.results[core]["s"]                      # [3, 8192] f32
            # H: u[0][p, j*32+h] -> (dd=p//8, h, w=4*(p%8)+j)
            UH = u[0].reshape(8, 8, 4, 32, 512).transpose(0, 3, 1, 2, 4) \
                .reshape(DS, H, W, 512)
            sH = s[0].reshape(4, 32, 8, 8).transpose(2, 1, 3, 0) \
                .reshape(DS, H, W)
            # W: u[1][p, j*32+w] -> (dd=p//8, h=4*(p%8)+j, w)
            UW = u[1].reshape(8, 32, 32, 512)
            sW = s[1].reshape(4, 32, 8, 8).transpose(2, 3, 0, 1) \
                .reshape(DS, H, W)
            # D: u[2] rows (p, j, dd) with p=h*8+wp, w=4*wp+j
            UD = u[2].reshape(32, 8, 4, 8, 512).transpose(3, 0, 1, 2, 4) \
                .reshape(DS, H, W, 512)
            sD = s[2].reshape(4, 8, 32, 8).transpose(1, 2, 3, 0) \
                .reshape(DS, H, W)
            sig = sH + sW + sD
            tot = (UH + UW + UD) / sig[..., None]
            acc[g * DS:(g + 1) * DS] = tot
        y[b] = x[b] + gam * acc.transpose(3, 1, 2, 0)
    return y
